# revision 2
# baseline (speedup 1.0000x reference)
"""Trainium2 Bass kernel for 3x3 conv (stride 1, pad 1) + bias.

Problem: x (32,128,56,56) f32, weights (256,128,3,3) f32, bias (256,) f32
         -> out (32,256,56,56) f32.

Strategy: data-parallel over batch (4 images per core, 8 cores).
Per core: implicit GEMM. C_in=128 lives on the SBUF partition axis (the
matmul contraction dim). Each image is stored width+height zero-padded
(58x58 grid) in a flat per-image slot so that, for every 3x3 tap (kh,kw),
the conv becomes ONE shifted contiguous matmul over 8 output rows
(N = 8*58 = 464) accumulated in PSUM across the 9 taps. C_out=256 is
split into two 128-partition halves (the matmul M dim). Bias is added
during PSUM->SBUF eviction on the scalar engine.

Inputs are converted to bf16 on the host (fp32 matmul is 1/4 rate on
TRN2's PE; bf16 streams 1 row/cycle and accumulates in fp32 PSUM).
"""

import os
from contextlib import ExitStack

import ml_dtypes
import numpy as np

import concourse.bacc as bacc
import concourse.bass as bass
import concourse.mybir as mybir
import concourse.tile as tile
import concourse.bass_utils as bass_utils

N_CORES = 8
B, CIN, H, W = 32, 128, 56, 56
COUT = 256
BPC = B // N_CORES          # images per core
PW, PH = W + 2, H + 2       # padded grid 58x58
GRID = PW * PH              # 3364
SLOT = GRID + 2             # per-image slot; grid lives at [1, 1+GRID)
RPC = 8                     # output rows per PSUM chunk
NCHUNK = H // RPC           # 7
NFREE = RPC * PW            # 464 moving-dim elements per matmul
KK = 9                      # 3x3 taps

DT = mybir.dt.bfloat16
NPDT = ml_dtypes.bfloat16

_CACHE: dict = {}


def _build():
    """Build the per-core Bass program (same program on all 8 cores)."""
    nc = bacc.Bacc("TRN2", target_bir_lowering=False, debug=False,
                   num_devices=N_CORES)
    f32 = mybir.dt.float32
    xp = nc.dram_tensor("xp", [BPC, CIN, SLOT], DT, kind="ExternalInput").ap()
    wt = nc.dram_tensor("wt", [CIN, KK * COUT], DT, kind="ExternalInput").ap()
    b2 = nc.dram_tensor("b2", [2, 128, 1], f32, kind="ExternalInput").ap()
    out = nc.dram_tensor("out", [BPC, COUT, H, W], f32,
                         kind="ExternalOutput").ap()

    with tile.TileContext(nc) as tc, ExitStack() as ctx:
        const_pool = ctx.enter_context(tc.tile_pool(name="const", bufs=1))
        xpool = ctx.enter_context(tc.tile_pool(name="xp_pool", bufs=1))
        opool = ctx.enter_context(tc.tile_pool(name="opool", bufs=8))
        psum = ctx.enter_context(
            tc.tile_pool(name="psum", bufs=8, space="PSUM"))

        wbuf = const_pool.tile([CIN, KK * COUT], DT)
        nc.sync.dma_start(wbuf[:], wt[:])
        bbuf = const_pool.tile([128, 2], f32)
        for h in range(2):
            nc.sync.dma_start(bbuf[:, h:h + 1], b2[h])

        xbuf = xpool.tile([CIN, BPC * SLOT], DT)
        for n in range(BPC):
            nc.sync.dma_start(xbuf[:, n * SLOT:(n + 1) * SLOT], xp[n])

        for n in range(BPC):
            base = n * SLOT
            for h in range(2):
                ot = opool.tile([128, H * W], f32, name="ot")
                for c in range(NCHUNK):
                    ps = psum.tile([128, NFREE], f32, name="ps")
                    for k in range(KK):
                        kh, kw = divmod(k, 3)
                        off = base + PW * (RPC * c + kh) + kw
                        nc.tensor.matmul(
                            ps[:],
                            wbuf[:, k * COUT + h * 128:
                                 k * COUT + h * 128 + 128],
                            xbuf[:, off:off + NFREE],
                            start=(k == 0),
                            stop=(k == KK - 1),
                        )
                    # evict: drop the width-padding columns, add bias
                    ps3 = ps.rearrange("p (r w) -> p r w", r=RPC)
                    ot3 = ot[:, c * RPC * W:(c + 1) * RPC * W].rearrange(
                        "p (r w) -> p r w", r=RPC)
                    nc.scalar.activation(
                        ot3[:], ps3[:, :, 1:57],
                        mybir.ActivationFunctionType.Identity,
                        bias=bbuf[:, h:h + 1])
                nc.sync.dma_start(
                    out[n, h * 128:(h + 1) * 128].rearrange(
                        "c h w -> c (h w)"),
                    ot[:])
    nc.compile()
    return nc


def _prep(x, weights, bias):
    """Host-side reshape/pad/cast into the device layouts."""
    xpad = np.zeros((B, CIN, SLOT), dtype=NPDT)
    grid = xpad[:, :, 1:1 + GRID].reshape(B, CIN, PH, PW)
    grid[:, :, 1:1 + H, 1:1 + W] = np.asarray(x).astype(NPDT)
    # weights (co, ci, kh, kw) -> (ci, kh*kw*co) flat
    wt = np.ascontiguousarray(
        np.asarray(weights).transpose(1, 2, 3, 0)).reshape(
            CIN, KK * COUT).astype(NPDT)
    b2 = np.asarray(bias).astype(np.float32).reshape(2, 128, 1)
    return xpad, wt, b2


def kernel(x, weights, bias):
    if "nc" not in _CACHE:
        _CACHE["nc"] = _build()
    nc = _CACHE["nc"]
    xpad, wt, b2 = _prep(x, weights, bias)
    in_maps = [
        {"xp": xpad[i * BPC:(i + 1) * BPC], "wt": wt, "b2": b2}
        for i in range(N_CORES)
    ]
    res = bass_utils.run_bass_kernel_spmd(
        nc, in_maps, core_ids=list(range(N_CORES)),
        trace=bool(int(os.environ.get("CONV_TRACE", "0"))),
    )
    if os.environ.get("CONV_TRACE"):
        _CACHE["last_result"] = res
    return np.concatenate([r["out"] for r in res.results], axis=0)


# revision 3
# speedup vs baseline: 1.0728x; 1.0728x over previous
"""Trainium2 Bass kernel for 3x3 conv (stride 1, pad 1) + bias.

Problem: x (32,128,56,56) f32, weights (256,128,3,3) f32, bias (256,) f32
         -> out (32,256,56,56) f32.

Strategy: data-parallel over batch (4 images per core, 8 cores).
Per core: implicit GEMM. C_in=128 lives on the SBUF partition axis (the
matmul contraction dim). Each image is stored width+height zero-padded
(58x58 grid) in a flat per-image slot so that, for every 3x3 tap (kh,kw),
the conv becomes ONE shifted contiguous matmul over 8 output rows
(N = 8*58 = 464) accumulated in PSUM across the 9 taps. C_out=256 is
split into two 128-partition halves (the matmul M dim). Bias is added
during PSUM->SBUF eviction on the scalar engine.

Inputs are converted to bf16 on the host (fp32 matmul is 1/4 rate on
TRN2's PE; bf16 streams 1 row/cycle and accumulates in fp32 PSUM).
"""

import os
from contextlib import ExitStack

import ml_dtypes
import numpy as np

import concourse.bacc as bacc
import concourse.bass as bass
import concourse.mybir as mybir
import concourse.tile as tile
import concourse.bass_utils as bass_utils

N_CORES = 8
B, CIN, H, W = 32, 128, 56, 56
COUT = 256
BPC = B // N_CORES          # images per core
PW, PH = W + 2, H + 2       # padded grid 58x58
GRID = PW * PH              # 3364
SLOT = GRID + 2             # per-image slot; grid lives at [1, 1+GRID)
RPC = 8                     # output rows per PSUM chunk
NCHUNK = H // RPC           # 7
NFREE = RPC * PW            # 464 moving-dim elements per matmul
KK = 9                      # 3x3 taps

DT = mybir.dt.bfloat16
NPDT = ml_dtypes.bfloat16

_CACHE: dict = {}


def _build():
    """Build the per-core Bass program (same program on all 8 cores)."""
    nc = bacc.Bacc("TRN2", target_bir_lowering=False, debug=False,
                   num_devices=N_CORES)
    f32 = mybir.dt.float32
    xp = nc.dram_tensor("xp", [BPC, CIN, SLOT], DT, kind="ExternalInput").ap()
    wt = nc.dram_tensor("wt", [CIN, KK * COUT], DT, kind="ExternalInput").ap()
    b2 = nc.dram_tensor("b2", [2, 128, 1], f32, kind="ExternalInput").ap()
    out = nc.dram_tensor("out", [BPC, COUT, H, W], f32,
                         kind="ExternalOutput").ap()

    with tile.TileContext(nc) as tc, ExitStack() as ctx:
        const_pool = ctx.enter_context(tc.tile_pool(name="const", bufs=1))
        xpool = ctx.enter_context(tc.tile_pool(name="xp_pool", bufs=1))
        epool = ctx.enter_context(tc.tile_pool(name="epool", bufs=6))
        psum = ctx.enter_context(
            tc.tile_pool(name="psum", bufs=8, space="PSUM"))

        wbuf = const_pool.tile([CIN, KK * COUT], DT)
        xbuf = xpool.tile([CIN, BPC * SLOT], DT)
        bbuf = const_pool.tile([128, 2], f32)

        # DMA-in order tuned so the first chunk's operands land first:
        # the first taps' weights, then image 0 in quarters (all-at-once
        # loads contend on the DMA engines and delay the first matmul).
        nc.sync.dma_start(wbuf[:, :2 * COUT], wt[:, :2 * COUT])
        q = SLOT // 4
        for piece in range(4):
            lo, hi = piece * q, (piece + 1) * q if piece < 3 else SLOT
            nc.sync.dma_start(xbuf[:, lo:hi], xp[0][:, lo:hi])
        nc.sync.dma_start(wbuf[:, 2 * COUT:], wt[:, 2 * COUT:])
        for h in range(2):
            nc.sync.dma_start(bbuf[:, h:h + 1], b2[h])
        hs = SLOT // 2
        for n in range(1, BPC):
            for lo, hi in ((0, hs), (hs, SLOT)):
                nc.sync.dma_start(
                    xbuf[:, n * SLOT + lo:n * SLOT + hi],
                    xp[n][:, lo:hi])

        for n in range(BPC):
            base = n * SLOT
            for h in range(2):
                for c in range(NCHUNK):
                    ps = psum.tile([128, NFREE], f32, name="ps")
                    for k in range(KK):
                        kh, kw = divmod(k, 3)
                        off = base + PW * (RPC * c + kh) + kw
                        nc.tensor.matmul(
                            ps[:],
                            wbuf[:, k * COUT + h * 128:
                                 k * COUT + h * 128 + 128],
                            xbuf[:, off:off + NFREE],
                            start=(k == 0),
                            stop=(k == KK - 1),
                        )
                    # evict: drop the width-padding columns, add bias
                    ps3 = ps.rearrange("p (r w) -> p r w", r=RPC)
                    ev = epool.tile([128, RPC * W], f32, name="ev")
                    nc.scalar.activation(
                        ev.rearrange("p (r w) -> p r w", r=RPC)[:],
                        ps3[:, :, 1:57],
                        mybir.ActivationFunctionType.Identity,
                        bias=bbuf[:, h:h + 1])
                    nc.sync.dma_start(
                        out[n, h * 128:(h + 1) * 128,
                            c * RPC:(c + 1) * RPC].rearrange(
                                "c r w -> c (r w)"),
                        ev[:])
    nc.compile()
    return nc


def _prep(x, weights, bias):
    """Host-side reshape/pad/cast into the device layouts."""
    xpad = np.zeros((B, CIN, SLOT), dtype=NPDT)
    grid = xpad[:, :, 1:1 + GRID].reshape(B, CIN, PH, PW)
    grid[:, :, 1:1 + H, 1:1 + W] = np.asarray(x).astype(NPDT)
    # weights (co, ci, kh, kw) -> (ci, kh*kw*co) flat
    wt = np.ascontiguousarray(
        np.asarray(weights).transpose(1, 2, 3, 0)).reshape(
            CIN, KK * COUT).astype(NPDT)
    b2 = np.asarray(bias).astype(np.float32).reshape(2, 128, 1)
    return xpad, wt, b2


def kernel(x, weights, bias):
    if "nc" not in _CACHE:
        _CACHE["nc"] = _build()
    nc = _CACHE["nc"]
    xpad, wt, b2 = _prep(x, weights, bias)
    in_maps = [
        {"xp": xpad[i * BPC:(i + 1) * BPC], "wt": wt, "b2": b2}
        for i in range(N_CORES)
    ]
    res = bass_utils.run_bass_kernel_spmd(
        nc, in_maps, core_ids=list(range(N_CORES)),
        trace=bool(int(os.environ.get("CONV_TRACE", "0"))),
    )
    if os.environ.get("CONV_TRACE"):
        _CACHE["last_result"] = res
    return np.concatenate([r["out"] for r in res.results], axis=0)
